# revision 3
# baseline (speedup 1.0000x reference)
"""GroupedQueryAttention Trainium2 kernel (8 NeuronCores) — pipelined v4.

Sharding: core c -> (batch b = c//4, kv-group g = c%4).
Each core computes its group's 4 query heads over its batch; the four
cores of a batch AllGather ctx^T per column strip (all 4 heads batched
into ONE collective), and each computes a 512-column slice of the
output projection (tensor-parallel along d_out).

v4 processes q in five uneven strips [256, 256, 512, 512, 512] so the
first (small) strips finish early and the serialized collective chain
starts ~25us sooner; collectives and the output projection overlap
later strips' attention compute.

Layout trick (unchanged): scores are computed transposed (S^T[k, q]) so
A^T = exp(S^T) is directly the lhsT of the ctx matmul; the softmax
denominator is a 129th "ones" column appended to V; normalization is a
per-partition scale of ctx[q, :] after the ctx matmul.
"""

from contextlib import ExitStack

import numpy as np
import ml_dtypes

import concourse.bass as bass
import concourse.bacc as bacc
import concourse.tile as tile
from concourse import mybir
from concourse.bass_utils import run_bass_kernel_spmd
from concourse.masks import make_identity
from concourse.tile_rust import add_dep_helper

BF16 = mybir.dt.bfloat16
F32 = mybir.dt.float32

B = 2
S = 2048
D = 2048
G = 4  # kv groups
HPG = 4  # heads per group
HD = 128  # head dim
NKT = S // 128  # 16 k-tiles
NDC = D // 128  # 16 d_in chunks
SCALE = 1.0 / float(np.sqrt(HD))
N_CORES = 8
REPLICA_GROUPS = [[0, 1, 2, 3], [4, 5, 6, 7]]

# (col_start, width) q-strips; first strips small so the collective
# chain starts early. Widths are multiples of 128, <= 512 (PSUM bank).
STRIPS = [(0, 256), (256, 512), (768, 512), (1280, 512), (1792, 256)]
NS = len(STRIPS)


def _build_program():
    nc = bacc.Bacc("TRN2", target_bir_lowering=False, debug=True)

    xq = nc.declare_dram_parameter("xq", [NDC, 128, S], BF16, isOutput=False)
    wq = nc.declare_dram_parameter("wq", [NDC, 128, HPG * HD], BF16, isOutput=False)
    wk = nc.declare_dram_parameter("wk", [NDC, 128, HD], BF16, isOutput=False)
    wv = nc.declare_dram_parameter("wv", [NDC, 128, HD], BF16, isOutput=False)
    wo = nc.declare_dram_parameter("wo", [NDC, 128, 512], BF16, isOutput=False)
    bo = nc.declare_dram_parameter("bo", [1, 512], BF16, isOutput=False)
    msk = nc.declare_dram_parameter("msk", [128, 896], BF16, isOutput=False)
    out_ext = nc.declare_dram_parameter("out", [S, 512], F32, isOutput=True)

    # AllGather outputs (Shared address space), one per strip: each
    # gathers all 4 heads' ctx^T strip from the 4 cores of this batch.
    gathq = [
        nc.dram_tensor(f"gathq{si}", [G, HPG, HD, w], BF16)
        for si, (c0, w) in enumerate(STRIPS)
    ]

    with tile.TileContext(nc) as tc, ExitStack() as es:
        singles = es.enter_context(tc.tile_pool(name="singles", bufs=1))
        wpool = es.enter_context(tc.tile_pool(name="w", bufs=1))
        xpool = es.enter_context(tc.tile_pool(name="x", bufs=2))
        qkpool = es.enter_context(tc.tile_pool(name="qk", bufs=1))
        apool = es.enter_context(tc.tile_pool(name="a", bufs=36))
        spool = es.enter_context(tc.tile_pool(name="sm", bufs=4))
        cpool = es.enter_context(tc.tile_pool(name="cs", bufs=6))
        ps_proj = es.enter_context(tc.tile_pool(name="psp", bufs=2, space="PSUM"))
        ps_small = es.enter_context(tc.tile_pool(name="pss", bufs=4, space="PSUM"))
        ps_out = es.enter_context(tc.tile_pool(name="pso", bufs=2, space="PSUM"))
        dram = es.enter_context(tc.tile_pool(name="dram", bufs=1, space="DRAM"))

        # --- constants ---
        ident = singles.tile([128, 128], BF16, tag="ident")
        make_identity(nc, ident)
        ones1 = singles.tile([1, 128], BF16, tag="ones1")
        nc.vector.memset(ones1, 1.0)
        bo_sb = singles.tile([1, 512], BF16, tag="bo")
        nc.scalar.dma_start(out=bo_sb, in_=bo[:, :])
        mask_sb = singles.tile([128, 896], BF16, tag="mask")
        nc.scalar.dma_start(out=mask_sb, in_=msk[:, :])

        # --- wk then first x strip so the K projection starts ASAP;
        # wq/wo on the Activation hwdge queue (parallel with SP) ---
        wkall = wpool.tile([128, NDC, HD], BF16, tag="wkall")
        nc.sync.dma_start(out=wkall, in_=wk.rearrange("a p q -> p a q"))
        xstrip0 = xpool.tile([128, NDC, 512], BF16, tag="xs")
        nc.sync.dma_start(
            out=xstrip0[:, :, 0 : STRIPS[0][1]],
            in_=xq[:, :, 0 : STRIPS[0][1]].rearrange("a p q -> p a q"),
        )
        wvall = wpool.tile([128, NDC, HD], BF16, tag="wvall")
        nc.sync.dma_start(out=wvall, in_=wv.rearrange("a p q -> p a q"))
        wqall = wpool.tile([128, NDC, HPG * HD], BF16, tag="wqall")
        nc.scalar.dma_start(out=wqall, in_=wq.rearrange("a p q -> p a q"))
        woall = wpool.tile([128, NDC, 512], BF16, tag="woall")
        nc.scalar.dma_start(out=woall, in_=wo.rearrange("a p q -> p a q"))
        wq_sb = [wqall[:, dc, :] for dc in range(NDC)]
        wk_sb = [wkall[:, dc, :] for dc in range(NDC)]
        wv_sb = [wvall[:, dc, :] for dc in range(NDC)]
        wo_sb = [woall[:, dc, :] for dc in range(NDC)]

        # --- persistent activations ---
        qT = [qkpool.tile([128, S], BF16, tag=f"qT{h}", name=f"qT{h}") for h in range(HPG)]
        kT = qkpool.tile([128, S], BF16, tag="kT")
        vext = [
            qkpool.tile([128, HD + 1], BF16, tag=f"v{i}", name=f"v{i}")
            for i in range(NKT)
        ]

        colls = [None] * NS
        ct_dmas = [None] * NS  # per-strip list of the 4 ct-write DMA handles

        def outproj(ss, after_si):
            """Output rows [c0, c0+w), this core's 512 columns.

            `after_si`: anchor the (collective-gated) cstrip reads AFTER the
            ct writes of attention strip `after_si` in the in-order SP DMA
            queue, so their semaphore wait can't head-of-line-block them.
            """
            c0, w = STRIPS[ss]
            nst = w // 128
            cstrip = spool.tile([128, G * HPG, w], BF16, tag="cstrip", bufs=2)
            # per-peer-group reads so matmuls start after the first chunk
            for g in range(G):
                d = nc.sync.dma_start(
                    out=cstrip[:, g * HPG : (g + 1) * HPG, :],
                    in_=gathq[ss][g].rearrange("h p q -> p h q"),
                )
                # shadow-memory tracking of plain (non-pool) DRAM tensors is
                # uncertain; make the read explicitly wait for the AllGather.
                add_dep_helper(d.ins, colls[ss].ins, reason="gather->read")
                for cd in ct_dmas[after_si]:
                    add_dep_helper(d.ins, cd.ins, reason="queue-order anchor")
            for half in range((nst + 1) // 2):  # 2 PSUM banks at a time
                sts = [2 * half + i for i in range(2) if 2 * half + i < nst]
                ops = {}
                for st in sts:
                    ps = ps_out.tile([128, 512], F32, tag="out")
                    # bias via K=1 matmul: out += ones^T @ bo
                    nc.tensor.matmul(ps, lhsT=ones1, rhs=bo_sb, start=True, stop=False)
                    ops[st] = ps
                for j in range(G * HPG):  # j = 4*g + h; ctx dim block = j
                    last = j == G * HPG - 1
                    for st in sts:
                        nc.tensor.matmul(
                            ops[st],
                            lhsT=cstrip[:, j, st * 128 : (st + 1) * 128],
                            rhs=wo_sb[j],
                            start=False,
                            stop=last,
                        )
                for st in sts:
                    osb = spool.tile([128, 512], F32, tag="osb")
                    if st % 2 == 0:
                        nc.vector.tensor_copy(osb, ops[st])
                    else:
                        nc.scalar.copy(osb, ops[st])
                    nc.sync.dma_start(
                        out=out_ext[
                            c0 + st * 128 : c0 + (st + 1) * 128, :
                        ],
                        in_=osb,
                    )

        for si, (c0, w) in enumerate(STRIPS):
            nst = w // 128  # s-tiles in this strip
            kt0 = c0 // 128  # first k-tile of this strip
            # ---- projections for strip si ----
            if si == 0:
                xstrip = xstrip0
            else:
                xstrip = xpool.tile([128, NDC, 512], BF16, tag="xs")
                nc.sync.dma_start(
                    out=xstrip[:, :, 0:w],
                    in_=xq[:, :, c0 : c0 + w].rearrange("a p q -> p a q"),
                )
            xs = [xstrip[:, dc, 0:w] for dc in range(NDC)]
            # K^T: [dh, q w]
            ps = ps_proj.tile([128, w], F32, tag="proj")
            for dc in range(NDC):
                nc.tensor.matmul(
                    ps,
                    lhsT=wk_sb[dc],
                    rhs=xs[dc],
                    start=(dc == 0),
                    stop=(dc == NDC - 1),
                )
            nc.vector.tensor_copy(kT[:, c0 : c0 + w], ps)
            # V: [s-tile 128, dv 128] + ones column
            for st in range(nst):
                kt = kt0 + st
                ps = ps_small.tile([128, HD + 1], F32, tag="small", bufs=2)
                for dc in range(NDC):
                    nc.tensor.matmul(
                        ps[:, 0:HD],
                        lhsT=xs[dc][:, st * 128 : (st + 1) * 128],
                        rhs=wv_sb[dc],
                        start=(dc == 0),
                        stop=(dc == NDC - 1),
                    )
                nc.vector.tensor_copy(vext[kt][:, 0:HD], ps[:, 0:HD])
                nc.vector.memset(vext[kt][:, HD : HD + 1], 1.0)

            # ---- attention for strip si, heads in pairs (hides exp latency);
            # each head's Q projection immediately precedes its scores ----
            nkt = kt0 + nst  # causal: k-tiles 0 .. kt0+nst-1
            ctq_dram = dram.tile([HPG, HD, w], BF16, tag=f"ctq{si}")
            ct_dmas[si] = []
            for hp in range(2):
                heads = (2 * hp, 2 * hp + 1)
                a_blocks = {h: [] for h in heads}
                for h in heads:
                    # Q^T for head h: [dh=128, q w]
                    ps = ps_proj.tile([128, w], F32, tag="proj")
                    for dc in range(NDC):
                        nc.tensor.matmul(
                            ps,
                            lhsT=wq_sb[dc][:, h * HD : (h + 1) * HD],
                            rhs=xs[dc],
                            start=(dc == 0),
                            stop=(dc == NDC - 1),
                        )
                    nc.vector.tensor_copy(qT[h][:, c0 : c0 + w], ps)
                    for kt in range(nkt):
                        ps = ps_proj.tile([128, w], F32, tag="proj")
                        nc.tensor.matmul(
                            ps,
                            lhsT=kT[:, kt * 128 : (kt + 1) * 128],
                            rhs=qT[h][:, c0 : c0 + w],
                            start=True,
                            stop=True,
                        )
                        a = apool.tile([128, w], BF16, tag="a")
                        nc.scalar.activation(
                            out=a,
                            in_=ps,
                            func=mybir.ActivationFunctionType.Exp,
                            scale=SCALE,
                        )
                        if kt * 128 + 127 > c0:  # diag block: mask (post-exp)
                            off = 128 * kt - c0
                            nc.vector.tensor_mul(
                                a, a, mask_sb[:, 384 - off : 384 - off + w]
                            )
                        a_blocks[h].append(a)
                for h in heads:
                    ctf = cpool.tile([128, 512], BF16, tag="ct")
                    ct = ctf[:, 0:w]
                    for st in range(nst):
                        qt = kt0 + st
                        cps = ps_small.tile([128, HD + 1], F32, tag="small", bufs=2)
                        for kt in range(qt + 1):
                            nc.tensor.matmul(
                                cps,
                                lhsT=a_blocks[h][kt][:, st * 128 : (st + 1) * 128],
                                rhs=vext[kt],
                                start=(kt == 0),
                                stop=(kt == qt),
                            )
                        zr = cpool.tile([128, 1], F32, tag="zr")
                        nc.vector.reciprocal(zr, cps[:, HD : HD + 1])
                        cs = cpool.tile([128, HD], BF16, tag="cs")
                        nc.vector.tensor_scalar_mul(cs, cps[:, 0:HD], zr)
                        tp = ps_small.tile([128, 128], BF16, tag="tp", bufs=2)
                        nc.tensor.transpose(tp, cs, ident)
                        nc.vector.tensor_copy(ct[:, st * 128 : (st + 1) * 128], tp)
                    ct_dmas[si].append(nc.sync.dma_start(out=ctq_dram[h], in_=ct))

            # ---- batched AllGather of all 4 heads' ctx^T for this strip ----
            colls[si] = nc.gpsimd.collective_compute(
                "AllGather",
                mybir.AluOpType.bypass,
                replica_groups=REPLICA_GROUPS,
                ins=[ctq_dram[:, :, :].opt()],
                outs=[gathq[si][:, :, :, :].opt()],
            )

            if si >= 2:
                outproj(si - 2, after_si=si)

        outproj(NS - 2, after_si=NS - 1)
        outproj(NS - 1, after_si=NS - 1)

    nc.compile()
    return nc


def _make_mask() -> np.ndarray:
    # base[k, j] = 1.0 if (j - 384) >= k else 0; diag block with offset
    # `off` uses columns [384-off : 384-off+w]: mask[k, q'] = (q' >= k + off).
    j = np.arange(896)[None, :]
    k = np.arange(128)[:, None]
    return ((j - 384) >= k).astype(ml_dtypes.bfloat16)


def _make_in_maps(inputs) -> list[dict]:
    x = np.asarray(inputs["x"], dtype=np.float32)
    Wq = np.asarray(inputs["Wq"], dtype=np.float32)
    Wk = np.asarray(inputs["Wk"], dtype=np.float32)
    Wv = np.asarray(inputs["Wv"], dtype=np.float32)
    Wo = np.asarray(inputs["Wo"], dtype=np.float32)
    bo = np.asarray(inputs["bo"], dtype=np.float32)

    bf = ml_dtypes.bfloat16
    mask = _make_mask()

    # x^T tiled by d-chunk: [dc, 128, S] per batch
    xqs = []
    for b in range(B):
        xT = np.ascontiguousarray(x[b].T.astype(bf))  # [d, s]
        xqs.append(np.ascontiguousarray(xT.reshape(NDC, 128, S)))

    in_maps = []
    for c in range(N_CORES):
        b, g = c // 4, c % 4
        in_maps.append(
            {
                "xq": xqs[b],
                "wq": np.ascontiguousarray(
                    Wq[:, g * 512 : (g + 1) * 512].astype(bf).reshape(NDC, 128, 512)
                ),
                "wk": np.ascontiguousarray(
                    Wk[:, g * HD : (g + 1) * HD].astype(bf).reshape(NDC, 128, HD)
                ),
                "wv": np.ascontiguousarray(
                    Wv[:, g * HD : (g + 1) * HD].astype(bf).reshape(NDC, 128, HD)
                ),
                "wo": np.ascontiguousarray(
                    Wo[:, g * 512 : (g + 1) * 512].astype(bf).reshape(NDC, 128, 512)
                ),
                "bo": np.ascontiguousarray(
                    bo[g * 512 : (g + 1) * 512].astype(bf).reshape(1, 512)
                ),
                "msk": mask,
            }
        )
    return in_maps


def _assemble(results) -> np.ndarray:
    out = np.empty((B, S, D), dtype=np.float32)
    for c in range(N_CORES):
        b, g = c // 4, c % 4
        out[b][:, g * 512 : (g + 1) * 512] = results[c]["out"]
    return out


def kernel(**inputs) -> np.ndarray:
    in_maps = _make_in_maps(inputs)
    nc = _build_program()
    res = run_bass_kernel_spmd(nc, in_maps, list(range(N_CORES)))
    return _assemble(res.results)
